# revision 1
# baseline (speedup 1.0000x reference)
"""BitNetLinear (ternary-quantized linear w/ training-blend) on 8 TRN2 NeuronCores.

Reference computation (fp32):
    thr  = mean(|W|)                       (global scalar over the full W)
    q    = sign(W) * (|W| > thr)           (ternary quantization)
    eff  = (1-l)*W + l*q, l=0.5            = 0.5*(W + q)
    eff  = eff * alpha
    out  = x @ eff^T + bias                x:[4,2048,4096] W:[4096,4096]

Sharding: tensor-parallel over out_features. Core c owns W rows
[c*512,(c+1)*512). x is replicated (pre-transposed to K-major bf16 on host),
the W shard is shipped K-major in fp32 (the threshold compare must see exact
fp32 values).

Two device phases (an on-device ncfw AllReduce measurably slows every
concurrent matmul ~20%, so the cross-core scalar reduction is done by
summing the 8 per-core partial outputs on the host instead — that sum is
just the unshard step of phase 1's reduce-scattered output):
  phase 1: each core reduces sum(|W_shard|) -> one fp32 scalar out.
  phase 2: takes the global sum as an input scalar; quantizes+blends the
    shard (fp32 math, bf16 effT cached in SBUF, [K,O] layout), streams
    x^T tiles, 2048 bf16 matmuls/core with fp32 PSUM accumulation, adds
    bias, writes the [8192, 512] fp32 output shard.
Host concatenates the 8 shards along the output-feature axis.
"""

import sys
import types

import numpy as np
import ml_dtypes


def _ensure_axon_hooks():
    """This image's antenv package lacks the axon_hooks submodule that
    concourse.bass_utils imports when tracing is requested (e.g. BASS_TRACE=1
    in the environment). Register a minimal stand-in so that path degrades
    gracefully instead of crashing."""
    try:
        import antenv.axon_hooks  # noqa: F401
        return
    except ImportError:
        pass
    try:
        import antenv
    except ImportError:
        return
    mod = types.ModuleType("antenv.axon_hooks")
    holder = {"hook": None}
    mod.set_axon_ntff_profile_hook = lambda h: holder.__setitem__("hook", h)
    mod.get_axon_ntff_profile_hook = lambda: holder["hook"]
    sys.modules["antenv.axon_hooks"] = mod
    antenv.axon_hooks = mod


_ensure_axon_hooks()

import concourse.bass as bass
import concourse.mybir as mybir
import concourse.tile as tile
from concourse import bacc
from concourse.bass_isa import ReduceOp
from concourse.bass_utils import run_bass_kernel_spmd

N_CORES = 8
CORE_IDS = list(range(N_CORES))

B, S, D_IN, D_OUT = 4, 2048, 4096, 4096
M = B * S                     # 8192 rows of x
O_SH = D_OUT // N_CORES       # 512 output features per core

P = 128                       # SBUF partitions
KO = D_IN // P                # 32 k-subtiles of 128
QCH = 4                       # k-subtiles per quantize chunk
NCH = KO // QCH               # 8 chunks
MT = 512                      # m-tile (x rows per output tile)
MS = MT // P                  # 4 PSUM subtiles per m-tile
NMT = M // MT                 # 16 m-tiles

_NC1 = None
_NC2 = None


def _build_phase1():
    """Per-core partial sum of |W_shard| -> [1,1] fp32.

    fp32 input: bf16 would halve the DMA but measures a systematic -2.2e-6
    relative bias on sum|w| (vs jnp's fp32 mean at ~3e-8), which moves the
    quantization threshold enough to flip ~35 mask elements and triple the
    absmax error. Not worth the ~5us.
    """
    dt = mybir.dt
    alu = mybir.AluOpType
    nc = bacc.Bacc("TRN2", target_bir_lowering=False, debug=False,
                   num_devices=N_CORES)
    wT = nc.dram_tensor("wT", [D_IN, O_SH], dt.float32, kind="ExternalInput").ap()
    psum_out = nc.dram_tensor("psum_out", [1, 1], dt.float32,
                              kind="ExternalOutput").ap()
    wT_r = wT.rearrange("(ko p) o -> p ko o", p=P)
    with tile.TileContext(nc) as tc:
        with (
            tc.tile_pool(name="persist", bufs=1) as persist,
            tc.tile_pool(name="wstage", bufs=4) as wstage,
        ):
            pp = persist.tile([P, KO], dt.float32)
            for g in range(NCH):
                wch = wstage.tile([P, QCH, O_SH], dt.float32, tag="wst",
                                  name=f"wch{g}")
                nc.sync.dma_start(wch[:], wT_r[:, g * QCH:(g + 1) * QCH, :])
                nc.vector.tensor_reduce(
                    pp[:, g * QCH:(g + 1) * QCH], wch[:],
                    axis=mybir.AxisListType.X, op=alu.add,
                    apply_absolute_value=True)
            part1 = persist.tile([P, 1], dt.float32)
            nc.vector.tensor_reduce(part1[:], pp[:], axis=mybir.AxisListType.X,
                                    op=alu.add)
            red = persist.tile([P, 1], dt.float32)
            nc.gpsimd.partition_all_reduce(red[:], part1[:], P, ReduceOp.add)
            nc.sync.dma_start(psum_out[:], red[0:1, :])
    nc.compile()
    return nc


def _build_phase2():
    dt = mybir.dt
    alu = mybir.AluOpType
    nc = bacc.Bacc("TRN2", target_bir_lowering=False, debug=False,
                   num_devices=N_CORES)

    xT = nc.dram_tensor("xT", [D_IN, M], dt.bfloat16, kind="ExternalInput").ap()
    wT = nc.dram_tensor("wT", [D_IN, O_SH], dt.float32, kind="ExternalInput").ap()
    bias_s = nc.dram_tensor("bias_s", [O_SH], dt.float32, kind="ExternalInput").ap()
    alpha_in = nc.dram_tensor("alpha_in", [1], dt.float32, kind="ExternalInput").ap()
    tot_in = nc.dram_tensor("tot_in", [1], dt.float32, kind="ExternalInput").ap()
    out = nc.dram_tensor("out", [M, O_SH], dt.float32, kind="ExternalOutput").ap()

    wT_r = wT.rearrange("(ko p) o -> p ko o", p=P)              # [128, 32, 512]
    xT_r = xT.rearrange("(ko p) m -> p ko m", p=P)              # [128, 32, 8192]
    out_r = out.rearrange("(mt ms p) o -> mt p ms o", p=P, ms=MS)

    with tile.TileContext(nc) as tc:
        with (
            tc.tile_pool(name="persist", bufs=1) as persist,
            tc.tile_pool(name="wstage", bufs=2) as wstage,
            tc.tile_pool(name="kxmp", bufs=3) as kxmp,
            tc.tile_pool(name="outp", bufs=3) as outp,
            tc.tile_pool(name="psum", bufs=2, space="PSUM") as psum,
        ):
            # ---- runtime scalars, broadcast per-partition ----
            # partition-broadcasts via K=1 PE matmul (ones[1,P].T @ row):
            # keeps GpSimd (and its slow library-reload) off the critical path
            # and gives the PE a head start.
            alpha_sb = persist.tile([1, 1], dt.float32)
            nc.sync.dma_start(alpha_sb[:], alpha_in[None, :])
            tot_sb = persist.tile([1, 1], dt.float32)
            nc.sync.dma_start(tot_sb[:], tot_in[None, :])
            sc_row = persist.tile([1, 4], dt.float32)
            nc.vector.memset(sc_row[:], 0.0)
            # sc_row = [c, thr, -thr, 0]
            nc.vector.tensor_scalar_mul(sc_row[:, 0:1], alpha_sb[:], 0.5)
            nc.vector.tensor_scalar_mul(sc_row[:, 1:2], tot_sb[:],
                                        1.0 / (D_OUT * D_IN))
            nc.vector.tensor_scalar_mul(sc_row[:, 2:3], sc_row[:, 1:2], -1.0)
            ones1 = persist.tile([1, P], dt.float32)
            nc.vector.memset(ones1[:], 1.0)
            psc = psum.tile([P, 4], dt.float32, tag="ps0", name="psc")
            nc.tensor.matmul(psc[:], ones1[:], sc_row[:], start=True, stop=True)
            sc_bc = persist.tile([P, 4], dt.float32)
            nc.vector.tensor_copy(sc_bc[:], psc[:])
            c_p = sc_bc[:, 0:1]
            thr_p = sc_bc[:, 1:2]
            negthr_p = sc_bc[:, 2:3]

            bias_row = persist.tile([1, O_SH], dt.float32)
            nc.sync.dma_start(bias_row[:], bias_s[None, :])
            pbias = psum.tile([P, O_SH], dt.float32, tag="ps1", name="pbias")
            nc.tensor.matmul(pbias[:], ones1[:], bias_row[:], start=True,
                             stop=True)
            bias_bc = persist.tile([P, O_SH], dt.float32)
            nc.vector.tensor_copy(bias_bc[:], pbias[:])

            # ---- quantize + blend -> effT bf16 [K, O] cached in SBUF ----
            # eff = c*(w + q), q = (sign(w-thr) + sign(w+thr)) / 2
            # (equivalent to (w>thr)-(w<-thr) except at exact fp32 ties,
            # which have ~zero probability; w-thr is exact near the
            # threshold by Sterbenz). The two sign passes run on the
            # otherwise-idle Scalar engine so DVE only does the 3 combine
            # passes — quantize throughput paces the first m-tiles.
            effT = persist.tile([P, KO, O_SH], dt.bfloat16)
            # ladder: small first chunks so the first matmuls start early;
            # steady chunks of 4 k-subtiles once the PE stream is rolling
            chunks = [1, 1, 2] + [QCH] * ((KO - 4) // QCH)
            assert sum(chunks) == KO
            pos = 0
            for g, ch in enumerate(chunks):
                sl = slice(pos, pos + ch)
                pos += ch
                wch = wstage.tile([P, QCH, O_SH], dt.float32, tag="wst",
                                  name=f"wch{g}")[:, :ch, :]
                nc.sync.dma_start(wch[:], wT_r[:, sl, :])
                s1 = wstage.tile([P, QCH, O_SH], dt.float32, tag="s1",
                                 name=f"s1_{g}", bufs=2)[:, :ch, :]
                nc.scalar.activation(s1[:], wch[:],
                                     mybir.ActivationFunctionType.Sign,
                                     bias=negthr_p[:])
                s2 = wstage.tile([P, QCH, O_SH], dt.float32, tag="s2",
                                 name=f"s2_{g}", bufs=2)[:, :ch, :]
                nc.scalar.activation(s2[:], wch[:],
                                     mybir.ActivationFunctionType.Sign,
                                     bias=thr_p[:])
                nc.vector.tensor_tensor(s1[:], s1[:], s2[:], alu.add)
                nc.vector.scalar_tensor_tensor(
                    out=s2[:], in0=s1[:], scalar=0.5, in1=wch[:],
                    op0=alu.mult, op1=alu.add)
                nc.vector.tensor_scalar_mul(effT[:, sl, :], s2[:], c_p[:])

            # ---- main matmul stream: out[m, o] = sum_k x[m,k] * eff[o,k] ----
            # m-tiles 0,1 run ksub-major across all 8 PSUM banks so the PE
            # consumes effT chunks at the rate the DVE quantize produces them
            pair = (0, 1)
            kxms = {}
            for mt in pair:
                kxm = kxmp.tile([P, KO, MT], dt.bfloat16, tag="kxm",
                                name=f"kxm{mt}")
                msl = slice(mt * MT, (mt + 1) * MT)
                for g in range(NCH):
                    nc.sync.dma_start(
                        kxm[:, g * QCH:(g + 1) * QCH, :],
                        xT_r[:, g * QCH:(g + 1) * QCH, msl])
                kxms[mt] = kxm
            ppts = {mt: [psum.tile([P, O_SH], dt.float32, tag=f"ps{j}",
                                   name=f"ps{j}_{mt}") for j in range(MS)]
                    for mt in pair}
            for ko in range(KO):
                for mt in pair:
                    for j in range(MS):
                        nc.tensor.matmul(
                            ppts[mt][j][:],
                            kxms[mt][:, ko, j * P:(j + 1) * P],
                            effT[:, ko, :],
                            start=(ko == 0), stop=(ko == KO - 1))
            for mt in pair:
                ot = outp.tile([P, MS, O_SH], dt.float32, tag="ot",
                               name=f"ot{mt}")
                for j in range(MS):
                    nc.vector.tensor_tensor(ot[:, j, :], ppts[mt][j][:],
                                            bias_bc[:], alu.add)
                nc.sync.dma_start(out_r[mt], ot[:])

            for mt in range(2, NMT):
                kxm = kxmp.tile([P, KO, MT], dt.bfloat16, tag="kxm",
                                name=f"kxm{mt}")
                msl = slice(mt * MT, (mt + 1) * MT)
                for g in range(NCH):
                    nc.sync.dma_start(
                        kxm[:, g * QCH:(g + 1) * QCH, :],
                        xT_r[:, g * QCH:(g + 1) * QCH, msl])
                pts = [psum.tile([P, O_SH], dt.float32, tag=f"ps{j}",
                                 name=f"ps{j}_{mt}") for j in range(MS)]
                for ko in range(KO):
                    for j in range(MS):
                        nc.tensor.matmul(
                            pts[j][:],
                            kxm[:, ko, j * P:(j + 1) * P],
                            effT[:, ko, :],
                            start=(ko == 0), stop=(ko == KO - 1))
                ot = outp.tile([P, MS, O_SH], dt.float32, tag="ot",
                               name=f"ot{mt}")
                for j in range(MS):
                    nc.vector.tensor_tensor(ot[:, j, :], pts[j][:], bias_bc[:],
                                            alu.add)
                    if mt == NMT - 1:
                        # finer stores at the end shorten the kernel tail
                        nc.sync.dma_start(out_r[mt][:, j, :], ot[:, j, :])
                if mt != NMT - 1:
                    nc.sync.dma_start(out_r[mt], ot[:])

    nc.compile()
    return nc


def _get_ncs():
    global _NC1, _NC2
    if _NC1 is None:
        _NC1 = _build_phase1()
    if _NC2 is None:
        _NC2 = _build_phase2()
    return _NC1, _NC2


def kernel(x: np.ndarray, weight_fp: np.ndarray, bias: np.ndarray,
           alpha: np.ndarray, _trace: bool = False, **_kw):
    x = np.asarray(x)
    weight_fp = np.asarray(weight_fp, dtype=np.float32)
    bias = np.asarray(bias, dtype=np.float32)
    alpha = np.asarray(alpha, dtype=np.float32)

    # host-side layout prep: x -> K-major bf16 (replicated), W shard -> K-major fp32
    x2 = np.ascontiguousarray(
        x.reshape(M, D_IN).astype(ml_dtypes.bfloat16).T)       # [D_IN, M]
    wshards = [np.ascontiguousarray(weight_fp[c * O_SH:(c + 1) * O_SH, :].T)
               for c in range(N_CORES)]                        # [D_IN, O_SH]

    nc1, nc2 = _get_ncs()

    # phase 1: per-core partial sums of |W|
    in1 = [{"wT": wshards[c]} for c in range(N_CORES)]
    res1 = run_bass_kernel_spmd(nc1, in1, CORE_IDS, trace=_trace)
    total = np.float32(sum(np.float64(res1.results[c]["psum_out"][0, 0])
                           for c in range(N_CORES)))

    # phase 2: quantize + matmul
    in2 = []
    for c in range(N_CORES):
        in2.append({
            "xT": x2,
            "wT": wshards[c],
            "bias_s": np.ascontiguousarray(bias[c * O_SH:(c + 1) * O_SH]),
            "alpha_in": alpha,
            "tot_in": np.array([total], dtype=np.float32),
        })
    res2 = run_bass_kernel_spmd(nc2, in2, CORE_IDS, trace=_trace)
    shards = [res2.results[c]["out"] for c in range(N_CORES)]
    full = np.concatenate(shards, axis=1).reshape(B, S, D_OUT)
    if _trace:
        kernel.last_exec_time_ns = (res1.exec_time_ns or 0) + (res2.exec_time_ns or 0)
        kernel.last_phase_times = (res1.exec_time_ns, res2.exec_time_ns)
    return full


if __name__ == "__main__":
    rng = np.random.default_rng(0)
    x = rng.standard_normal((B, S, D_IN), dtype=np.float32)
    w = rng.standard_normal((D_OUT, D_IN), dtype=np.float32)
    b = np.zeros(D_OUT, np.float32)
    a = np.ones(1, np.float32)
    out = kernel(x, w, b, a)
    print("out", out.shape, out.dtype, out[0, 0, :4])



# revision 2
# speedup vs baseline: 1.0227x; 1.0227x over previous
"""BitNetLinear (ternary-quantized linear w/ training-blend) on 8 TRN2 NeuronCores.

Reference computation (fp32):
    thr  = mean(|W|)                       (global scalar over the full W)
    q    = sign(W) * (|W| > thr)           (ternary quantization)
    eff  = (1-l)*W + l*q, l=0.5            = 0.5*(W + q)
    eff  = eff * alpha
    out  = x @ eff^T + bias                x:[4,2048,4096] W:[4096,4096]

Sharding: tensor-parallel over out_features. Core c owns W rows
[c*512,(c+1)*512). x is replicated (pre-transposed to K-major bf16 on host),
the W shard is shipped K-major in fp32 (the threshold compare must see exact
fp32 values).

Two device phases. A measured 4-byte on-device AllReduce costs ~65us for the
first op in a NEFF (+~9us marginal), far more than the whole phase-1 NEFF, so
the cross-core scalar reduction stays host-side: phase 1 reduce-scatters the
|W| sum (one fp32 partial per core) and the host sum of 8 scalars is the
unshard step.
  phase 1: each core reduces sum(|W_shard|) -> one fp32 scalar out. The
    16-chunk DMA stream is ungated (bufs=16) so it runs at full HBM rate.
  phase 2: takes the global sum as an input scalar. The startup is DMA-bound,
    so W chunks and the first two x-tiles are interleaved in issue order and
    the quantize ladder follows W-chunk arrival; the PE stream starts ~13us
    in (vs ~36us when quantize was serialized behind a buffer-gated W load).
    effT caches the UNSCALED blend (w+q) in bf16; the 0.5*alpha scale rides
    the bias stage (one scalar_tensor_tensor per PSUM tile: out = c*psum + b),
    which drops one DVE pass per quantize chunk. Dummy fp32 matmuls gated on
    W chunk 0 warm the PE HAM clock gate (cold PE runs at 1.2 GHz for the
    first ~3.4us of activity). The last m-tile runs j-major so its stores
    drain during the final matmuls instead of after them.
Host concatenates the 8 [8192, 512] output shards along the feature axis.
"""

import sys
import types

import numpy as np
import ml_dtypes


def _ensure_axon_hooks():
    """This image's antenv package lacks the axon_hooks submodule that
    concourse.bass_utils imports when tracing is requested (e.g. BASS_TRACE=1
    in the environment). Register a minimal stand-in so that path degrades
    gracefully instead of crashing."""
    try:
        import antenv.axon_hooks  # noqa: F401
        return
    except ImportError:
        pass
    try:
        import antenv
    except ImportError:
        return
    mod = types.ModuleType("antenv.axon_hooks")
    holder = {"hook": None}
    mod.set_axon_ntff_profile_hook = lambda h: holder.__setitem__("hook", h)
    mod.get_axon_ntff_profile_hook = lambda: holder["hook"]
    sys.modules["antenv.axon_hooks"] = mod
    antenv.axon_hooks = mod


_ensure_axon_hooks()

import concourse.bass as bass
import concourse.mybir as mybir
import concourse.tile as tile
from concourse import bacc
from concourse.bass_isa import ReduceOp
from concourse.bass_utils import run_bass_kernel_spmd

N_CORES = 8
CORE_IDS = list(range(N_CORES))

B, S, D_IN, D_OUT = 4, 2048, 4096, 4096
M = B * S                     # 8192 rows of x
O_SH = D_OUT // N_CORES       # 512 output features per core
P = 128                       # SBUF partitions
KO = D_IN // P                # 32 k-subtiles of 128
WCH = 2                       # k-subtiles per W DMA/quantize chunk
NWCH = KO // WCH              # 16 chunks
XG = 4                        # k-subtiles per x-tile DMA group
NXG = KO // XG                # 8 groups
MT = 512                      # m-tile (x rows per output tile)
MS = MT // P                  # 4 PSUM subtiles per m-tile
NMT = M // MT                 # 16 m-tiles

_NC1 = None
_NC2 = None


def _build_phase1():
    """Per-core partial sum of |W_shard| -> [1,1] fp32.

    fp32 input: bf16 would halve the DMA but measures a systematic -2.2e-6
    relative bias on sum|w| (vs jnp's fp32 mean at ~3e-8), which moves the
    quantization threshold enough to flip ~35 mask elements and triple the
    absmax error. Not worth the ~5us.
    """
    dt = mybir.dt
    alu = mybir.AluOpType
    nc = bacc.Bacc("TRN2", target_bir_lowering=False, debug=False,
                   num_devices=N_CORES)
    wT = nc.dram_tensor("wT", [D_IN, O_SH], dt.float32, kind="ExternalInput").ap()
    psum_out = nc.dram_tensor("psum_out", [1, 1], dt.float32,
                              kind="ExternalOutput").ap()
    wT_r = wT.rearrange("(ko p) o -> p ko o", p=P)
    with tile.TileContext(nc) as tc:
        with (
            tc.tile_pool(name="persist", bufs=1) as persist,
            # bufs == chunk count: every chunk DMA issues ungated, so the
            # 8 MB stream runs at the full HBM share (~22us) instead of
            # being serialized behind the per-chunk reduce.
            tc.tile_pool(name="wstage", bufs=NWCH) as wstage,
        ):
            pp = persist.tile([P, KO], dt.float32)
            wchs = []
            for g in range(NWCH):
                wch = wstage.tile([P, WCH, O_SH], dt.float32, tag="wst",
                                  name=f"wch{g}")
                nc.sync.dma_start(wch[:], wT_r[:, g * WCH:(g + 1) * WCH, :])
                wchs.append(wch)
            for g in range(NWCH):
                nc.vector.tensor_reduce(
                    pp[:, g * WCH:(g + 1) * WCH], wchs[g][:],
                    axis=mybir.AxisListType.X, op=alu.add,
                    apply_absolute_value=True)
            part1 = persist.tile([P, 1], dt.float32)
            nc.vector.tensor_reduce(part1[:], pp[:], axis=mybir.AxisListType.X,
                                    op=alu.add)
            red = persist.tile([P, 1], dt.float32)
            nc.gpsimd.partition_all_reduce(red[:], part1[:], P, ReduceOp.add)
            nc.sync.dma_start(psum_out[:], red[0:1, :])
    nc.compile()
    return nc


def _build_phase2():
    dt = mybir.dt
    alu = mybir.AluOpType
    act = mybir.ActivationFunctionType
    nc = bacc.Bacc("TRN2", target_bir_lowering=False, debug=False,
                   num_devices=N_CORES)

    xT = nc.dram_tensor("xT", [D_IN, M], dt.bfloat16, kind="ExternalInput").ap()
    wT = nc.dram_tensor("wT", [D_IN, O_SH], dt.float32, kind="ExternalInput").ap()
    bias_s = nc.dram_tensor("bias_s", [O_SH], dt.float32, kind="ExternalInput").ap()
    alpha_in = nc.dram_tensor("alpha_in", [1], dt.float32, kind="ExternalInput").ap()
    tot_in = nc.dram_tensor("tot_in", [1], dt.float32, kind="ExternalInput").ap()
    out = nc.dram_tensor("out", [M, O_SH], dt.float32, kind="ExternalOutput").ap()

    wT_r = wT.rearrange("(ko p) o -> p ko o", p=P)              # [128, 32, 512]
    xT_r = xT.rearrange("(ko p) m -> p ko m", p=P)              # [128, 32, 8192]
    out_r = out.rearrange("(mt ms p) o -> mt p ms o", p=P, ms=MS)

    with tile.TileContext(nc) as tc:
        with (
            tc.tile_pool(name="persist", bufs=1) as persist,
            tc.tile_pool(name="wstage", bufs=8) as wstage,
            tc.tile_pool(name="sstage", bufs=2) as sstage,
            tc.tile_pool(name="kxmp", bufs=3) as kxmp,
            tc.tile_pool(name="outp", bufs=3) as outp,
            tc.tile_pool(name="psum", bufs=2, space="PSUM") as psum,
        ):
            # ---- tiny input DMAs + runtime scalars (in the launch shadow) ----
            alpha_sb = persist.tile([1, 1], dt.float32)
            nc.sync.dma_start(alpha_sb[:], alpha_in[None, :])
            tot_sb = persist.tile([1, 1], dt.float32)
            nc.sync.dma_start(tot_sb[:], tot_in[None, :])
            bias_row = persist.tile([1, O_SH], dt.float32)
            nc.sync.dma_start(bias_row[:], bias_s[None, :])

            sc_row = persist.tile([1, 4], dt.float32)
            nc.vector.memset(sc_row[:], 0.0)
            # sc_row = [c, thr, -thr, 0], c = 0.5*alpha
            nc.vector.tensor_scalar_mul(sc_row[:, 0:1], alpha_sb[:], 0.5)
            nc.vector.tensor_scalar_mul(sc_row[:, 1:2], tot_sb[:],
                                        1.0 / (D_OUT * D_IN))
            nc.vector.tensor_scalar_mul(sc_row[:, 2:3], sc_row[:, 1:2], -1.0)
            ones1 = persist.tile([1, P], dt.float32)
            nc.vector.memset(ones1[:], 1.0)

            # ---- bulk DMA issue order IS the priority order: W chunks feed
            # the quantize ladder (paces the first matmuls), the pair x-tiles
            # trail slightly, the steady x-tiles come last. ----
            wchs = []
            kxms = {}
            for mt in (0, 1):
                kxms[mt] = kxmp.tile([P, KO, MT], dt.bfloat16, tag="kxm",
                                     name=f"kxm{mt}")

            def dma_w(g):
                wch = wstage.tile([P, WCH, O_SH], dt.float32, tag="wst",
                                  name=f"wch{g}")
                nc.sync.dma_start(wch[:], wT_r[:, g * WCH:(g + 1) * WCH, :])
                wchs.append(wch)

            def dma_x(mt, g):
                msl = slice(mt * MT, (mt + 1) * MT)
                nc.sync.dma_start(
                    kxms[mt][:, g * XG:(g + 1) * XG, :],
                    xT_r[:, g * XG:(g + 1) * XG, msl])

            dma_w(0)
            dma_x(0, 0)
            dma_x(1, 0)
            g_w, g_x = 1, 1
            while g_w < NWCH or g_x < NXG:
                for _ in range(2):
                    if g_w < NWCH:
                        dma_w(g_w)
                        g_w += 1
                if g_x < NXG:
                    dma_x(0, g_x)
                    dma_x(1, g_x)
                    g_x += 1

            # ---- broadcasts via K=1 PE matmuls (ones[1,P].T @ row) ----
            pbias = psum.tile([P, O_SH], dt.float32, tag="ps1", name="pbias")
            nc.tensor.matmul(pbias[:], ones1[:], bias_row[:], start=True,
                             stop=True)
            bias_bc = persist.tile([P, O_SH], dt.float32)
            nc.vector.tensor_copy(bias_bc[:], pbias[:])

            psc = psum.tile([P, 4], dt.float32, tag="ps0", name="psc")
            nc.tensor.matmul(psc[:], ones1[:], sc_row[:], start=True, stop=True)
            sc_bc = persist.tile([P, 4], dt.float32)
            nc.vector.tensor_copy(sc_bc[:], psc[:])
            c_p = sc_bc[:, 0:1]
            thr_p = sc_bc[:, 1:2]
            negthr_p = sc_bc[:, 2:3]

            # ---- HAM warmup: dummy fp32 matmuls gated on W chunk 0. The PE
            # clock gate needs ~3.4us of sustained activity to lift 1.2 ->
            # 2.4 GHz; these burn the wait for the first effT chunk. ----
            for dtile in ("ps2", "ps3"):
                pdum = psum.tile([P, O_SH], dt.float32, tag=dtile,
                                 name=f"dum_{dtile}")
                for i in range(7):
                    nc.tensor.matmul(pdum[:], bias_bc[:, 0:P],
                                     wchs[0][:, 0, :], start=(i == 0),
                                     stop=(i == 6))

            # ---- quantize + blend -> effT bf16 [K, O] cached in SBUF ----
            # effT = w + q (UNSCALED), q = (sign(w-thr) + sign(w+thr)) / 2
            # (equivalent to (w>thr)-(w<-thr) except at exact fp32 ties,
            # which have ~zero probability; w-thr is exact near the
            # threshold by Sterbenz). Sign runs on the otherwise-idle Scalar
            # engine; DVE does 2 combine passes. The 0.5*alpha scale is
            # folded into the bias stage.
            effT = persist.tile([P, KO, O_SH], dt.bfloat16)
            # ladder: 1-ksub first chunks so the first matmuls start early
            qchunks = [1, 1] + [WCH] * ((KO - 2) // WCH)
            assert sum(qchunks) == KO
            pos = 0
            for qi, ch in enumerate(qchunks):
                sl = slice(pos, pos + ch)
                # W chunks are WCH ksubs; map ksub range -> staged tile slice
                gw = pos // WCH
                lo = pos - gw * WCH
                wch = wchs[gw][:, lo:lo + ch, :]
                pos += ch
                s1 = sstage.tile([P, WCH, O_SH], dt.float32, tag="s1",
                                 name=f"s1_{qi}")[:, :ch, :]
                nc.scalar.activation(s1[:], wch[:], act.Sign, bias=negthr_p[:])
                s2 = sstage.tile([P, WCH, O_SH], dt.float32, tag="s2",
                                 name=f"s2_{qi}")[:, :ch, :]
                nc.scalar.activation(s2[:], wch[:], act.Sign, bias=thr_p[:])
                nc.vector.tensor_tensor(s1[:], s1[:], s2[:], alu.add)
                # effT = 0.5*(s1+s2) + w  = q + w   (bf16 out)
                nc.vector.scalar_tensor_tensor(
                    out=effT[:, sl, :], in0=s1[:], scalar=0.5, in1=wch[:],
                    op0=alu.mult, op1=alu.add)

            # ---- main matmul stream: psum[m, o] = sum_k x[m,k] * effT[o,k],
            # out = c*psum + bias. m-tiles 0,1 run ksub-major across all 8
            # PSUM banks so the PE consumes effT at half the rate the
            # quantize produces it (jitter margin). ----
            pair = (0, 1)
            ppts = {mt: [psum.tile([P, O_SH], dt.float32, tag=f"ps{j}",
                                   name=f"ps{j}_{mt}") for j in range(MS)]
                    for mt in pair}
            for ko in range(KO):
                for mt in pair:
                    for j in range(MS):
                        nc.tensor.matmul(
                            ppts[mt][j][:],
                            kxms[mt][:, ko, j * P:(j + 1) * P],
                            effT[:, ko, :],
                            start=(ko == 0), stop=(ko == KO - 1))

            def finish_tile(mt, pts, per_j_store=False):
                """bias stage: ot = c*psum + bias, stores per half m-tile."""
                for h in range(2):
                    ot = outp.tile([P, 2, O_SH], dt.float32, tag="ot",
                                   name=f"ot{mt}_{h}")
                    for jj in range(2):
                        j = 2 * h + jj
                        nc.vector.scalar_tensor_tensor(
                            out=ot[:, jj, :], in0=pts[j][:], scalar=c_p[:],
                            in1=bias_bc[:], op0=alu.mult, op1=alu.add)
                        if per_j_store:
                            nc.sync.dma_start(out_r[mt][:, j, :], ot[:, jj, :])
                    if not per_j_store:
                        nc.sync.dma_start(out_r[mt][:, 2 * h:2 * h + 2, :],
                                          ot[:])

            for mt in pair:
                finish_tile(mt, ppts[mt])

            for mt in range(2, NMT):
                kxm = kxmp.tile([P, KO, MT], dt.bfloat16, tag="kxm",
                                name=f"kxm{mt}")
                msl = slice(mt * MT, (mt + 1) * MT)
                for g in range(NXG):
                    nc.sync.dma_start(
                        kxm[:, g * XG:(g + 1) * XG, :],
                        xT_r[:, g * XG:(g + 1) * XG, msl])
                pts = [psum.tile([P, O_SH], dt.float32, tag=f"ps{j}",
                                 name=f"ps{j}_{mt}") for j in range(MS)]
                last = mt == NMT - 1
                if last:
                    # j-major: each PSUM tile finishes 32 matmuls early, so
                    # bias+store drain during the remaining matmuls.
                    for j in range(MS):
                        for ko in range(KO):
                            nc.tensor.matmul(
                                pts[j][:],
                                kxm[:, ko, j * P:(j + 1) * P],
                                effT[:, ko, :],
                                start=(ko == 0), stop=(ko == KO - 1))
                    finish_tile(mt, pts, per_j_store=True)
                else:
                    for ko in range(KO):
                        for j in range(MS):
                            nc.tensor.matmul(
                                pts[j][:],
                                kxm[:, ko, j * P:(j + 1) * P],
                                effT[:, ko, :],
                                start=(ko == 0), stop=(ko == KO - 1))
                    finish_tile(mt, pts)

    nc.compile()
    return nc


def _get_ncs():
    global _NC1, _NC2
    if _NC1 is None:
        _NC1 = _build_phase1()
    if _NC2 is None:
        _NC2 = _build_phase2()
    return _NC1, _NC2


def kernel(x: np.ndarray, weight_fp: np.ndarray, bias: np.ndarray,
           alpha: np.ndarray, _trace: bool = False, **_kw):
    x = np.asarray(x)
    weight_fp = np.asarray(weight_fp, dtype=np.float32)
    bias = np.asarray(bias, dtype=np.float32)
    alpha = np.asarray(alpha, dtype=np.float32)

    # host-side layout prep: x -> K-major bf16 (replicated), W shard -> K-major fp32
    x2 = np.ascontiguousarray(
        x.reshape(M, D_IN).astype(ml_dtypes.bfloat16).T)       # [D_IN, M]
    wshards = [np.ascontiguousarray(weight_fp[c * O_SH:(c + 1) * O_SH, :].T)
               for c in range(N_CORES)]                        # [D_IN, O_SH]

    nc1, nc2 = _get_ncs()

    # phase 1: per-core partial sums of |W|
    in1 = [{"wT": wshards[c]} for c in range(N_CORES)]
    res1 = run_bass_kernel_spmd(nc1, in1, CORE_IDS, trace=_trace)
    total = np.float32(sum(np.float64(res1.results[c]["psum_out"][0, 0])
                           for c in range(N_CORES)))

    # phase 2: quantize + matmul
    in2 = []
    for c in range(N_CORES):
        in2.append({
            "xT": x2,
            "wT": wshards[c],
            "bias_s": np.ascontiguousarray(bias[c * O_SH:(c + 1) * O_SH]),
            "alpha_in": alpha,
            "tot_in": np.array([total], dtype=np.float32),
        })
    res2 = run_bass_kernel_spmd(nc2, in2, CORE_IDS, trace=_trace)
    shards = [res2.results[c]["out"] for c in range(N_CORES)]
    full = np.concatenate(shards, axis=1).reshape(B, S, D_OUT)
    if _trace:
        kernel.last_exec_time_ns = (res1.exec_time_ns or 0) + (res2.exec_time_ns or 0)
        kernel.last_phase_times = (res1.exec_time_ns, res2.exec_time_ns)
    return full


if __name__ == "__main__":
    rng = np.random.default_rng(0)
    x = rng.standard_normal((B, S, D_IN), dtype=np.float32)
    w = rng.standard_normal((D_OUT, D_IN), dtype=np.float32)
    b = np.zeros(D_OUT, np.float32)
    a = np.ones(1, np.float32)
    out = kernel(x, w, b, a)
    print("out", out.shape, out.dtype, out[0, 0, :4])


# revision 3
# speedup vs baseline: 1.0429x; 1.0197x over previous
"""BitNetLinear (ternary-quantized linear w/ training-blend) on 8 TRN2 NeuronCores.

Reference computation (fp32):
    thr  = mean(|W|)                       (global scalar over the full W)
    q    = sign(W) * (|W| > thr)           (ternary quantization)
    eff  = (1-l)*W + l*q, l=0.5            = 0.5*(W + q)
    eff  = eff * alpha
    out  = x @ eff^T + bias                x:[4,2048,4096] W:[4096,4096]

Sharding: tensor-parallel over out_features. Core c owns W rows
[c*512,(c+1)*512). x is replicated (pre-transposed to K-major bf16 on host),
the W shard is shipped K-major in fp32 (the threshold compare must see exact
fp32 values).

Two device phases. A measured 4-byte on-device AllReduce costs ~65us for the
first op in a NEFF (+~9us marginal), far more than the whole phase-1 NEFF, so
the cross-core scalar reduction stays host-side: phase 1 reduce-scatters the
|W| sum (one fp32 partial per core) and the host sum of 8 scalars is the
unshard step.
  phase 1: each core reduces sum(|W_shard|) -> one fp32 scalar out. The
    16-chunk DMA stream is ungated (bufs=16) so it runs at full HBM rate.
  phase 2: takes the global sum as an input scalar. The startup is DMA-bound,
    so W chunks and the first two x-tiles are interleaved in issue order and
    the quantize ladder follows W-chunk arrival; the PE stream starts ~13us
    in (vs ~36us when quantize was serialized behind a buffer-gated W load).
    effT caches the UNSCALED blend (w+q) in bf16; the 0.5*alpha scale rides
    the bias stage (one scalar_tensor_tensor per PSUM tile: out = c*psum + b),
    which drops one DVE pass per quantize chunk. Dummy fp32 matmuls gated on
    W chunk 0 warm the PE HAM clock gate (cold PE runs at 1.2 GHz for the
    first ~3.4us of activity). The last m-tile runs j-major so its stores
    drain during the final matmuls instead of after them.
Host concatenates the 8 [8192, 512] output shards along the feature axis.
"""

import sys
import types

import numpy as np
import ml_dtypes


def _ensure_axon_hooks():
    """This image's antenv package lacks the axon_hooks submodule that
    concourse.bass_utils imports when tracing is requested (e.g. BASS_TRACE=1
    in the environment). Register a minimal stand-in so that path degrades
    gracefully instead of crashing."""
    try:
        import antenv.axon_hooks  # noqa: F401
        return
    except ImportError:
        pass
    try:
        import antenv
    except ImportError:
        return
    mod = types.ModuleType("antenv.axon_hooks")
    holder = {"hook": None}
    mod.set_axon_ntff_profile_hook = lambda h: holder.__setitem__("hook", h)
    mod.get_axon_ntff_profile_hook = lambda: holder["hook"]
    sys.modules["antenv.axon_hooks"] = mod
    antenv.axon_hooks = mod


_ensure_axon_hooks()

import concourse.bass as bass
import concourse.mybir as mybir
import concourse.tile as tile
from concourse import bacc
from concourse.bass_isa import ReduceOp
from concourse.bass_utils import run_bass_kernel_spmd

N_CORES = 8
CORE_IDS = list(range(N_CORES))

B, S, D_IN, D_OUT = 4, 2048, 4096, 4096
M = B * S                     # 8192 rows of x
O_SH = D_OUT // N_CORES       # 512 output features per core
P = 128                       # SBUF partitions
KO = D_IN // P                # 32 k-subtiles of 128
WCH = 2                       # k-subtiles per W DMA/quantize chunk
NWCH = KO // WCH              # 16 chunks
XG = 4                        # k-subtiles per x-tile DMA group
NXG = KO // XG                # 8 groups
MT = 512                      # m-tile (x rows per output tile)
MS = MT // P                  # 4 PSUM subtiles per m-tile
NMT = M // MT                 # 16 m-tiles

_NC1 = None
_NC2 = None


def _build_phase1():
    """Per-core partial sum of |W_shard| -> [1,1] fp32.

    fp32 input: bf16 would halve the DMA but measures a systematic -2.2e-6
    relative bias on sum|w| (vs jnp's fp32 mean at ~3e-8), which moves the
    quantization threshold enough to flip ~35 mask elements and triple the
    absmax error. Not worth the ~5us.
    """
    dt = mybir.dt
    alu = mybir.AluOpType
    nc = bacc.Bacc("TRN2", target_bir_lowering=False, debug=False,
                   num_devices=N_CORES)
    wT = nc.dram_tensor("wT", [D_IN, O_SH], dt.float32, kind="ExternalInput").ap()
    psum_out = nc.dram_tensor("psum_out", [1, 1], dt.float32,
                              kind="ExternalOutput").ap()
    wT_r = wT.rearrange("(ko p) o -> p ko o", p=P)
    with tile.TileContext(nc) as tc:
        with (
            tc.tile_pool(name="persist", bufs=1) as persist,
            # bufs == chunk count: every chunk DMA issues ungated, so the
            # 8 MB stream runs at the full HBM share (~22us) instead of
            # being serialized behind the per-chunk reduce.
            tc.tile_pool(name="wstage", bufs=NWCH) as wstage,
        ):
            pp = persist.tile([P, KO], dt.float32)
            wchs = []
            for g in range(NWCH):
                wch = wstage.tile([P, WCH, O_SH], dt.float32, tag="wst",
                                  name=f"wch{g}")
                nc.sync.dma_start(wch[:], wT_r[:, g * WCH:(g + 1) * WCH, :])
                wchs.append(wch)
            for g in range(NWCH):
                nc.vector.tensor_reduce(
                    pp[:, g * WCH:(g + 1) * WCH], wchs[g][:],
                    axis=mybir.AxisListType.X, op=alu.add,
                    apply_absolute_value=True)
            part1 = persist.tile([P, 1], dt.float32)
            nc.vector.tensor_reduce(part1[:], pp[:], axis=mybir.AxisListType.X,
                                    op=alu.add)
            red = persist.tile([P, 1], dt.float32)
            nc.gpsimd.partition_all_reduce(red[:], part1[:], P, ReduceOp.add)
            nc.sync.dma_start(psum_out[:], red[0:1, :])
    nc.compile()
    return nc


def _build_phase2():
    dt = mybir.dt
    alu = mybir.AluOpType
    act = mybir.ActivationFunctionType
    nc = bacc.Bacc("TRN2", target_bir_lowering=False, debug=False,
                   num_devices=N_CORES)

    xT = nc.dram_tensor("xT", [D_IN, M], dt.bfloat16, kind="ExternalInput").ap()
    wT = nc.dram_tensor("wT", [D_IN, O_SH], dt.float32, kind="ExternalInput").ap()
    bias_s = nc.dram_tensor("bias_s", [O_SH], dt.float32, kind="ExternalInput").ap()
    alpha_in = nc.dram_tensor("alpha_in", [1], dt.float32, kind="ExternalInput").ap()
    tot_in = nc.dram_tensor("tot_in", [1], dt.float32, kind="ExternalInput").ap()
    out = nc.dram_tensor("out", [M, O_SH], dt.float32, kind="ExternalOutput").ap()

    wT_r = wT.rearrange("(ko p) o -> p ko o", p=P)              # [128, 32, 512]
    xT_r = xT.rearrange("(ko p) m -> p ko m", p=P)              # [128, 32, 8192]
    out_r = out.rearrange("(mt ms p) o -> mt p ms o", p=P, ms=MS)

    with tile.TileContext(nc) as tc:
        with (
            tc.tile_pool(name="persist", bufs=1) as persist,
            tc.tile_pool(name="wstage", bufs=8) as wstage,
            tc.tile_pool(name="sstage", bufs=2) as sstage,
            tc.tile_pool(name="kxmp", bufs=3) as kxmp,
            tc.tile_pool(name="outp", bufs=3) as outp,
            tc.tile_pool(name="psum", bufs=2, space="PSUM") as psum,
        ):
            # ---- ungated vector init first: the warmup matmuls and the
            # broadcast matmuls need these, and any input-gated vector op
            # emitted earlier would head-block the queue. ----
            ones1 = persist.tile([1, P], dt.float32)
            nc.vector.memset(ones1[:], 1.0)
            scratch = persist.tile([P, O_SH], dt.bfloat16)
            nc.vector.memset(scratch[:], 0.5)

            # ---- tiny input DMAs + runtime scalars (in the launch shadow) ----
            alpha_sb = persist.tile([1, 1], dt.float32)
            nc.sync.dma_start(alpha_sb[:], alpha_in[None, :])
            tot_sb = persist.tile([1, 1], dt.float32)
            nc.sync.dma_start(tot_sb[:], tot_in[None, :])
            bias_row = persist.tile([1, O_SH], dt.float32)
            nc.sync.dma_start(bias_row[:], bias_s[None, :])

            sc_row = persist.tile([1, 4], dt.float32)
            nc.vector.memset(sc_row[:], 0.0)
            # sc_row = [c, thr, -thr, 0], c = 0.5*alpha
            nc.vector.tensor_scalar_mul(sc_row[:, 0:1], alpha_sb[:], 0.5)
            nc.vector.tensor_scalar_mul(sc_row[:, 1:2], tot_sb[:],
                                        1.0 / (D_OUT * D_IN))
            nc.vector.tensor_scalar_mul(sc_row[:, 2:3], sc_row[:, 1:2], -1.0)

            # ---- bulk DMA issue order IS the priority order: W chunks feed
            # the quantize ladder (paces the first matmuls), the pair x-tiles
            # trail slightly, the steady x-tiles come last. The first chunks
            # are small so the first matmul fires ~3us earlier. ----
            wchunks = [1, 1] + [WCH] * ((KO - 2) // WCH)     # 17 W chunks
            xgroups = [2, 2] + [XG] * ((KO - 4) // XG)       # 9 x groups
            assert sum(wchunks) == KO and sum(xgroups) == KO
            wpos = [0]
            for ch in wchunks:
                wpos.append(wpos[-1] + ch)
            xpos = [0]
            for ch in xgroups:
                xpos.append(xpos[-1] + ch)

            wchs = []
            kxms = {}
            for mt in (0, 1):
                kxms[mt] = kxmp.tile([P, KO, MT], dt.bfloat16, tag="kxm",
                                     name=f"kxm{mt}")

            def dma_w(g):
                ch = wchunks[g]
                wch = wstage.tile([P, WCH, O_SH], dt.float32, tag="wst",
                                  name=f"wch{g}")
                nc.sync.dma_start(wch[:, :ch, :],
                                  wT_r[:, wpos[g]:wpos[g + 1], :])
                wchs.append(wch)

            def dma_x(mt, g):
                msl = slice(mt * MT, (mt + 1) * MT)
                nc.sync.dma_start(
                    kxms[mt][:, xpos[g]:xpos[g + 1], :],
                    xT_r[:, xpos[g]:xpos[g + 1], msl])

            dma_w(0)
            dma_x(0, 0)
            dma_x(1, 0)
            g_w, g_x = 1, 1
            while g_w < len(wchunks) or g_x < len(xgroups):
                for _ in range(2):
                    if g_w < len(wchunks):
                        dma_w(g_w)
                        g_w += 1
                if g_x < len(xgroups):
                    dma_x(0, g_x)
                    dma_x(1, g_x)
                    g_x += 1

            # ---- HAM warmup around the broadcast matmuls: the PE clock gate
            # needs ~3.4us of sustained activity to lift 1.2 -> 2.4 GHz.
            # Batch 1 is ungated (scratch source) and starts as soon as the
            # engines come up; batch 2 bridges the gap to the first real
            # matmul so the pair stream starts at full clock. ----
            pdum = psum.tile([P, O_SH], dt.float32, tag="ps2", name="dum1")
            for i in range(5):
                nc.tensor.matmul(pdum[:], scratch[:, 0:P], scratch[:],
                                 start=(i == 0), stop=(i == 4))

            # broadcasts via K=1 PE matmuls (ones[1,P].T @ row)
            psc = psum.tile([P, 4], dt.float32, tag="ps0", name="psc")
            nc.tensor.matmul(psc[:], ones1[:], sc_row[:], start=True, stop=True)
            pbias = psum.tile([P, O_SH], dt.float32, tag="ps1", name="pbias")
            nc.tensor.matmul(pbias[:], ones1[:], bias_row[:], start=True,
                             stop=True)
            sc_bc = persist.tile([P, 4], dt.float32)
            nc.vector.tensor_copy(sc_bc[:], psc[:])
            bias_bc = persist.tile([P, O_SH], dt.float32)
            nc.vector.tensor_copy(bias_bc[:], pbias[:])
            c_p = sc_bc[:, 0:1]
            thr_p = sc_bc[:, 1:2]
            negthr_p = sc_bc[:, 2:3]

            pdum2 = psum.tile([P, O_SH], dt.float32, tag="ps3", name="dum2")
            for i in range(6):
                nc.tensor.matmul(pdum2[:], scratch[:, 0:P], scratch[:],
                                 start=(i == 0), stop=(i == 5))

            # ---- quantize + blend -> effT bf16 [K, O] cached in SBUF ----
            # effT = w + q (UNSCALED), q = (sign(w-thr) + sign(w+thr)) / 2
            # (equivalent to (w>thr)-(w<-thr) except at exact fp32 ties,
            # which have ~zero probability; w-thr is exact near the
            # threshold by Sterbenz). Sign runs on the otherwise-idle Scalar
            # engine; DVE does 2 combine passes. The 0.5*alpha scale is
            # folded into the bias stage. Quantize chunks are 1:1 with the
            # W DMA chunks.
            effT = persist.tile([P, KO, O_SH], dt.bfloat16)
            for qi, ch in enumerate(wchunks):
                sl = slice(wpos[qi], wpos[qi + 1])
                wch = wchs[qi][:, :ch, :]
                s1 = sstage.tile([P, WCH, O_SH], dt.float32, tag="s1",
                                 name=f"s1_{qi}")[:, :ch, :]
                nc.scalar.activation(s1[:], wch[:], act.Sign, bias=negthr_p[:])
                s2 = sstage.tile([P, WCH, O_SH], dt.float32, tag="s2",
                                 name=f"s2_{qi}")[:, :ch, :]
                nc.scalar.activation(s2[:], wch[:], act.Sign, bias=thr_p[:])
                nc.vector.tensor_tensor(s1[:], s1[:], s2[:], alu.add)
                # effT = 0.5*(s1+s2) + w  = q + w   (bf16 out)
                nc.vector.scalar_tensor_tensor(
                    out=effT[:, sl, :], in0=s1[:], scalar=0.5, in1=wch[:],
                    op0=alu.mult, op1=alu.add)

            # ---- main matmul stream: psum[m, o] = sum_k x[m,k] * effT[o,k],
            # out = c*psum + bias. m-tiles 0,1 run ksub-major across all 8
            # PSUM banks so the PE consumes effT at half the rate the
            # quantize produces it (jitter margin). ----
            pair = (0, 1)
            ppts = {mt: [psum.tile([P, O_SH], dt.float32, tag=f"ps{j}",
                                   name=f"ps{j}_{mt}") for j in range(MS)]
                    for mt in pair}
            for ko in range(KO):
                for mt in pair:
                    for j in range(MS):
                        nc.tensor.matmul(
                            ppts[mt][j][:],
                            kxms[mt][:, ko, j * P:(j + 1) * P],
                            effT[:, ko, :],
                            start=(ko == 0), stop=(ko == KO - 1))

            def finish_tile(mt, pts, per_j_store=False):
                """bias stage: ot = c*psum + bias, stores per half m-tile."""
                for h in range(2):
                    ot = outp.tile([P, 2, O_SH], dt.float32, tag="ot",
                                   name=f"ot{mt}_{h}")
                    for jj in range(2):
                        j = 2 * h + jj
                        nc.vector.scalar_tensor_tensor(
                            out=ot[:, jj, :], in0=pts[j][:], scalar=c_p[:],
                            in1=bias_bc[:], op0=alu.mult, op1=alu.add)
                        if per_j_store:
                            nc.sync.dma_start(out_r[mt][:, j, :], ot[:, jj, :])
                    if not per_j_store:
                        nc.sync.dma_start(out_r[mt][:, 2 * h:2 * h + 2, :],
                                          ot[:])

            for mt in pair:
                finish_tile(mt, ppts[mt])

            for mt in range(2, NMT):
                kxm = kxmp.tile([P, KO, MT], dt.bfloat16, tag="kxm",
                                name=f"kxm{mt}")
                msl = slice(mt * MT, (mt + 1) * MT)
                for g in range(NXG):
                    nc.sync.dma_start(
                        kxm[:, g * XG:(g + 1) * XG, :],
                        xT_r[:, g * XG:(g + 1) * XG, msl])
                pts = [psum.tile([P, O_SH], dt.float32, tag=f"ps{j}",
                                 name=f"ps{j}_{mt}") for j in range(MS)]
                last = mt == NMT - 1
                if last:
                    # j-major: each PSUM tile finishes 32 matmuls early, so
                    # bias+store drain during the remaining matmuls.
                    for j in range(MS):
                        for ko in range(KO):
                            nc.tensor.matmul(
                                pts[j][:],
                                kxm[:, ko, j * P:(j + 1) * P],
                                effT[:, ko, :],
                                start=(ko == 0), stop=(ko == KO - 1))
                    finish_tile(mt, pts, per_j_store=True)
                else:
                    for ko in range(KO):
                        for j in range(MS):
                            nc.tensor.matmul(
                                pts[j][:],
                                kxm[:, ko, j * P:(j + 1) * P],
                                effT[:, ko, :],
                                start=(ko == 0), stop=(ko == KO - 1))
                    finish_tile(mt, pts)

    nc.compile()
    return nc


def _get_ncs():
    global _NC1, _NC2
    if _NC1 is None:
        _NC1 = _build_phase1()
    if _NC2 is None:
        _NC2 = _build_phase2()
    return _NC1, _NC2


def kernel(x: np.ndarray, weight_fp: np.ndarray, bias: np.ndarray,
           alpha: np.ndarray, _trace: bool = False, **_kw):
    x = np.asarray(x)
    weight_fp = np.asarray(weight_fp, dtype=np.float32)
    bias = np.asarray(bias, dtype=np.float32)
    alpha = np.asarray(alpha, dtype=np.float32)

    # host-side layout prep: x -> K-major bf16 (replicated), W shard -> K-major fp32
    x2 = np.ascontiguousarray(
        x.reshape(M, D_IN).astype(ml_dtypes.bfloat16).T)       # [D_IN, M]
    wshards = [np.ascontiguousarray(weight_fp[c * O_SH:(c + 1) * O_SH, :].T)
               for c in range(N_CORES)]                        # [D_IN, O_SH]

    nc1, nc2 = _get_ncs()

    # phase 1: per-core partial sums of |W|
    in1 = [{"wT": wshards[c]} for c in range(N_CORES)]
    res1 = run_bass_kernel_spmd(nc1, in1, CORE_IDS, trace=_trace)
    total = np.float32(sum(np.float64(res1.results[c]["psum_out"][0, 0])
                           for c in range(N_CORES)))

    # phase 2: quantize + matmul
    in2 = []
    for c in range(N_CORES):
        in2.append({
            "xT": x2,
            "wT": wshards[c],
            "bias_s": np.ascontiguousarray(bias[c * O_SH:(c + 1) * O_SH]),
            "alpha_in": alpha,
            "tot_in": np.array([total], dtype=np.float32),
        })
    res2 = run_bass_kernel_spmd(nc2, in2, CORE_IDS, trace=_trace)
    shards = [res2.results[c]["out"] for c in range(N_CORES)]
    full = np.concatenate(shards, axis=1).reshape(B, S, D_OUT)
    if _trace:
        kernel.last_exec_time_ns = (res1.exec_time_ns or 0) + (res2.exec_time_ns or 0)
        kernel.last_phase_times = (res1.exec_time_ns, res2.exec_time_ns)
    return full


if __name__ == "__main__":
    rng = np.random.default_rng(0)
    x = rng.standard_normal((B, S, D_IN), dtype=np.float32)
    w = rng.standard_normal((D_OUT, D_IN), dtype=np.float32)
    b = np.zeros(D_OUT, np.float32)
    a = np.ones(1, np.float32)
    out = kernel(x, w, b, a)
    print("out", out.shape, out.dtype, out[0, 0, :4])
